# revision 2
# baseline (speedup 1.0000x reference)
"""Multi-head causal self-attention (D=768, H=12, S=4096) on 8 Trainium2 cores.

v2: q-major PV, fp8-DoubleRow Q/K projections, split exp (Act exact for
heads 0-1, DVE Schraudolph-bf16 for head 2), Pool int16 AND-masks,
DMA-transpose for the output-projection operand, sub-bank PSUM accumulators.

Sharding: 4 head-groups (3 heads) x 2 interleaved query-sets; core = 2g+s.
SPMD: one program, per-core behaviour via input data only.
"""

import numpy as np

D = 768
S = 4096
H = 12
HD = 64
NG = 4          # head groups
GH = 3          # heads per group
GD = GH * HD    # 192
SL = S // 2     # local queries per core
P = 128
NC = D // P     # 6 contraction chunks
QG = 4          # query groups per core
QGS = 512
NKB = S // P    # 32 key blocks

A_SCH = 128.0 / np.log(2.0)      # Schraudolph bf16 scale
B_SCH = 127.0 * 128.0 - 6.0      # bias, C=6 centering

_CACHE = {}


def _build_program():
    import concourse.bacc as bacc
    import concourse.mybir as mybir
    import concourse.tile as tile
    from contextlib import ExitStack

    bf16 = mybir.dt.bfloat16
    f32 = mybir.dt.float32
    fp8 = mybir.dt.float8e4
    i16 = mybir.dt.int16

    Exp = mybir.ActivationFunctionType.Exp
    Copy = mybir.ActivationFunctionType.Copy
    mult = mybir.AluOpType.mult
    add = mybir.AluOpType.add
    band = mybir.AluOpType.bitwise_and
    DR = mybir.MatmulPerfMode.DoubleRow

    nc = bacc.Bacc("TRN2", target_bir_lowering=False, debug=False, num_devices=8)

    xtb = nc.dram_tensor("xtb", [QG, P, NC * 1024], bf16, kind="ExternalInput").ap()
    xt8 = nc.dram_tensor("xt8", [QG, P, NC * 1024], fp8, kind="ExternalInput").ap()
    xq8 = nc.dram_tensor("xq8", [QG, P, NC * 512], fp8, kind="ExternalInput").ap()
    wq8 = nc.dram_tensor("wq8", [P, NC * GD], fp8, kind="ExternalInput").ap()
    wk8 = nc.dram_tensor("wk8", [P, NC * GD], fp8, kind="ExternalInput").ap()
    wvb = nc.dram_tensor("wvb", [P, NC * GD], bf16, kind="ExternalInput").ap()
    wo0 = nc.dram_tensor("wo0", [P, D], bf16, kind="ExternalInput").ap()
    wo1 = nc.dram_tensor("wo1", [65, D], bf16, kind="ExternalInput").ap()
    bq4 = nc.dram_tensor("bq4", [P, 4], f32, kind="ExternalInput").ap()
    mskp = nc.dram_tensor("mskp", [P, 2 * GH * P], bf16, kind="ExternalInput").ap()
    out = nc.dram_tensor("out", [SL, D], f32, kind="ExternalOutput").ap()
    import os as _os
    _dbg = _os.environ.get("KV2_DEBUG") == "1"
    if _dbg:
        dbg_qt = nc.dram_tensor("dbg_qt", [P, SL], bf16, kind="ExternalOutput").ap()
        dbg_kt = nc.dram_tensor("dbg_kt", [P, S], bf16, kind="ExternalOutput").ap()
        dbg_vv = nc.dram_tensor("dbg_vv", [P, GH * NKB * 65], bf16, kind="ExternalOutput").ap()
        dbg_s0 = nc.dram_tensor("dbg_s0", [P, QGS], bf16, kind="ExternalOutput").ap()
        dbg_s1 = nc.dram_tensor("dbg_s1", [P, QGS], bf16, kind="ExternalOutput").ap()
        dbg_pt = nc.dram_tensor("dbg_pt", [P, 2 * GH * QGS], bf16, kind="ExternalOutput").ap()
        dbg_den = nc.dram_tensor("dbg_den", [P, 12], f32, kind="ExternalOutput").ap()

    with tile.TileContext(nc) as tc, ExitStack() as ctx:
        const = ctx.enter_context(tc.tile_pool(name="const", bufs=1))

        xt_sb = const.tile([P, NC, S], bf16, tag="xt")
        xt8_sb = const.tile([P, NC, S], fp8, tag="xt8")
        xq8_sb = const.tile([P, NC, SL], fp8, tag="xq8")
        wq8_sb = const.tile([P, NC, GD], fp8, tag="wq8")
        wk8_sb = const.tile([P, NC, GD], fp8, tag="wk8")
        wv_sb = const.tile([P, NC, GD], bf16, tag="wv")
        wo0_sb = const.tile([P, D], bf16, tag="wo0")
        wo1_sb = const.tile([65, D], bf16, tag="wo1")
        bq_sb = const.tile([P, 4], f32, tag="bq")
        msk_sb = const.tile([P, 2, GH, P], bf16, tag="msk")
        kt01 = const.tile([P, S], bf16, tag="kt01")
        kt2 = const.tile([64, S], bf16, tag="kt2")
        qt01 = const.tile([P, SL], bf16, tag="qt01")
        qt2 = const.tile([64, SL], bf16, tag="qt2")
        vv = const.tile([P, GH, NKB, 65], bf16, tag="vv")
        stk0 = const.tile([P, QGS], bf16, tag="stk0")
        stk1 = const.tile([P, QGS], bf16, tag="stk1")

        # ---- input DMAs, priority order ----
        nc.sync.dma_start(wq8_sb[:], wq8[:].rearrange("p (c d) -> p c d", d=GD))
        nc.sync.dma_start(xq8_sb[:, :, 0:512],
                          xq8[0].rearrange("p (c d) -> p c d", d=512))
        nc.sync.dma_start(wk8_sb[:], wk8[:].rearrange("p (c d) -> p c d", d=GD))
        nc.sync.dma_start(xt8_sb[:, :, 0:1024],
                          xt8[0].rearrange("p (c d) -> p c d", d=1024))
        nc.sync.dma_start(wv_sb[:], wvb[:].rearrange("p (c d) -> p c d", d=GD))
        nc.sync.dma_start(xt_sb[:, :, 0:1024],
                          xtb[0].rearrange("p (c d) -> p c d", d=1024))
        nc.sync.dma_start(bq_sb[:], bq4[:])
        nc.sync.dma_start(msk_sb[:],
                          mskp[:].rearrange("p (a h q) -> p a h q", h=GH, q=P))
        nc.sync.dma_start(wo0_sb[:], wo0[:])
        nc.sync.dma_start(wo1_sb[:], wo1[:])
        for g in range(1, QG):
            nc.sync.dma_start(
                xq8_sb[:, :, g * 512:(g + 1) * 512],
                xq8[g].rearrange("p (c d) -> p c d", d=512))
            nc.sync.dma_start(
                xt8_sb[:, :, g * 1024:(g + 1) * 1024],
                xt8[g].rearrange("p (c d) -> p c d", d=1024))
            nc.sync.dma_start(
                xt_sb[:, :, g * 1024:(g + 1) * 1024],
                xtb[g].rearrange("p (c d) -> p c d", d=1024))
        nc.gpsimd.memset(vv[:], 1.0)         # ones column survives at [.., 64]

        kq = [(kt01, 0), (kt01, 64), (kt2, 0)]
        qq = [(qt01, 0), (qt01, 64), (qt2, 0)]

        # flattened block list: (qg, t, par), 80 blocks
        blocks = [(qg, t, par)
                  for qg in range(QG)
                  for t in range(4 * (qg + 1))
                  for par in (0, 1)]
        first_step = {}   # qg -> global step index of its first block
        for ii, (qg, t, par) in enumerate(blocks):
            if qg not in first_step:
                first_step[qg] = ii

        with tc.tile_pool(name="stA_ps", bufs=2, space="PSUM") as stA, \
             tc.tile_pool(name="stB_ps", bufs=2, space="PSUM") as stB, \
             tc.tile_pool(name="pv_ps", bufs=1, space="PSUM") as pvps, \
             tc.tile_pool(name="pt", bufs=2) as ptpool, \
             tc.tile_pool(name="sq", bufs=3) as sqpool, \
             tc.tile_pool(name="rc", bufs=4) as rcpool, \
             tc.tile_pool(name="oev", bufs=3) as oevpool:

            def proj_piece(g, idx):
                """Pieces 0..9 of projection chunk g (Q, K0, K1, V0..V3)."""
                if idx in (0, 1):
                    lo = idx == 0
                    ps = (stB.tile([P, QGS], f32, tag="stB", name=f"qA{g}")
                          if lo else
                          stB.tile([64, QGS], f32, tag="stB", name=f"qB{g}"))
                    dsl = slice(0, 128) if lo else slice(128, GD)
                    for j in range(3):
                        nc.tensor.matmul(
                            ps[:], wq8_sb[:, 2 * j:2 * j + 2, dsl],
                            xq8_sb[:, 2 * j:2 * j + 2, g * QGS:(g + 1) * QGS],
                            start=(j == 0), stop=(j == 2), perf_mode=DR)
                    if lo:
                        nc.vector.tensor_scalar(
                            qt01[:, g * QGS:(g + 1) * QGS], ps[:],
                            1.0 / 512.0, bq_sb[:, 0:1], mult, add)
                    else:
                        nc.vector.tensor_scalar(
                            qt2[:, g * QGS:(g + 1) * QGS], ps[:],
                            1.0 / 512.0, bq_sb[0:64, 1:2], mult, add)
                elif idx in (2, 3, 4, 5):
                    kg = 2 * g + (idx - 2) // 2
                    lo = (idx % 2) == 0
                    ps = (stB.tile([P, QGS], f32, tag="stB", name=f"kA{kg}")
                          if lo else
                          stB.tile([64, QGS], f32, tag="stB", name=f"kB{kg}"))
                    dsl = slice(0, 128) if lo else slice(128, GD)
                    for j in range(3):
                        nc.tensor.matmul(
                            ps[:], wk8_sb[:, 2 * j:2 * j + 2, dsl],
                            xt8_sb[:, 2 * j:2 * j + 2, kg * QGS:(kg + 1) * QGS],
                            start=(j == 0), stop=(j == 2), perf_mode=DR)
                    if lo:
                        nc.vector.tensor_scalar(
                            kt01[:, kg * QGS:(kg + 1) * QGS], ps[:],
                            1.0 / 64.0, bq_sb[:, 2:3], mult, add)
                    else:
                        nc.vector.tensor_scalar(
                            kt2[:, kg * QGS:(kg + 1) * QGS], ps[:],
                            1.0 / 64.0, bq_sb[0:64, 3:4], mult, add)
                else:
                    vp = idx - 6
                    kb0 = 8 * g + 2 * vp
                    psv = stB.tile([P, 2, GD], f32, tag="stB", name=f"v{kb0}")
                    for b in range(2):
                        for c in range(NC):
                            nc.tensor.matmul(
                                psv[:, b, :],
                                xt_sb[:, c, (kb0 + b) * P:(kb0 + b + 1) * P],
                                wv_sb[:, c, :],
                                start=(b == 0 and c == 0), stop=(c == NC - 1),
                                skip_group_check=True)
                    nc.vector.tensor_copy(
                        vv[:, :, kb0:kb0 + 2, 0:64],
                        psv[:].rearrange("p b (h d) -> p h b d", h=GH))

            pv_tiles = {}

            def emit_qk(step):
                if step >= len(blocks):
                    return None
                qg, t, par = blocks[step]
                q0 = P * max(0, t - 4 * qg)
                m = 2 * t + par
                stT = stA.tile([P, 2, QGS], f32, tag="stA",
                               name=f"sA{qg}_{m}")
                stH = stB.tile([P, QGS], f32, tag="stB", name=f"sB{qg}_{m}")
                for h in range(GH):
                    kt_t, kb_p = kq[h]
                    qt_t, qb_p = qq[h]
                    dst = stT[:, h, q0:] if h < 2 else stH[:, q0:]
                    nc.tensor.matmul(
                        dst,
                        kt_t[kb_p:kb_p + 64, m * P:(m + 1) * P],
                        qt_t[qb_p:qb_p + 64, qg * QGS + q0:(qg + 1) * QGS],
                        start=True, stop=True)
                return (stT, stH)

            def norm_piece(qg, jj):
                pvr = pv_tiles[qg]
                b, i0 = jj // 2, (jj % 2) * GH
                rc = rcpool.tile([P, GH, 1], f32, tag="rc",
                                 name=f"rc{qg}_{jj}")
                nc.vector.reciprocal(rc[:], pvr[:, b, i0:i0 + GH, 64:65])
                sq = sqpool.tile([P, 256], bf16, tag="sq",
                                 name=f"sq{qg}_{jj}")
                for h in range(GH):
                    nc.vector.tensor_scalar(
                        sq[:, h * 64:(h + 1) * 64],
                        pvr[:, b, i0 + h, 0:64], rc[:, h, :], None, mult)
                nc.vector.memset(sq[:, GD:GD + 1], 1.0)
                nc.sync.dma_start_transpose(
                    stk0[:, jj * P:(jj + 1) * P], sq[:, 0:128])
                nc.sync.dma_start_transpose(
                    stk1[:, jj * P:(jj + 1) * P], sq[:, 128:256])

            def op_piece(qg, jj):
                oe = oevpool.tile([P, D], f32, tag="oe", name=f"oe{qg}_{jj}")
                for half in range(2):
                    ps = stB.tile([P, QGS], f32, tag="stB",
                                  name=f"op{qg}_{jj}_{half}")
                    nc.tensor.matmul(
                        ps[:, 0:384],
                        stk0[:, jj * P:(jj + 1) * P],
                        wo0_sb[:, half * 384:(half + 1) * 384],
                        start=True, stop=False)
                    nc.tensor.matmul(
                        ps[:, 0:384],
                        stk1[0:65, jj * P:(jj + 1) * P],
                        wo1_sb[:, half * 384:(half + 1) * 384],
                        start=False, stop=True)
                    nc.scalar.activation(
                        oe[:, half * 384:(half + 1) * 384], ps[:, 0:384], Copy)
                jq = 4 * qg + jj
                nc.sync.dma_start(out[jq * P:(jq + 1) * P, :], oe[:])

            # per-step extra pieces: step index -> list of callables
            sched = {}

            def add_piece(step, fn):
                sched.setdefault(step, []).append(fn)

            # chunk 0: Q, K0, V0 in prologue; K1, V1..V3 early steps
            add_piece(0, lambda: proj_piece(0, 4))
            add_piece(0, lambda: proj_piece(0, 5))
            add_piece(0, lambda: proj_piece(0, 7))   # V1 (blocks 2,3)
            add_piece(1, lambda: proj_piece(0, 8))   # V2
            add_piece(2, lambda: proj_piece(0, 9))   # V3
            # chunk qg+1 spread over qg's steps; tail of qg at start of qg+1
            for qg in range(QG):
                base = first_step[qg]
                nsteps = 8 * (qg + 1)
                if qg + 1 < QG:
                    for i in range(10):
                        add_piece(base + 3 + (i * (nsteps - 4)) // 10,
                                  lambda g=qg + 1, k=i: proj_piece(g, k))
                nbase = first_step[qg + 1] if qg + 1 < QG else len(blocks)
                for jj in range(4):
                    if qg + 1 < QG:
                        add_piece(nbase + 2 * jj,
                                  lambda g=qg, j=jj: op_piece(g, j))
                    # norms handled at boundary below

            # ---- prologue ----
            for i in (0, 1, 2, 3, 6):     # Q-A, Q-B, K0-A, K0-B, V0
                proj_piece(0, i)
            sts = [emit_qk(0), emit_qk(1)]

            # ---- main pipeline ----
            for step, (qg, t, par) in enumerate(blocks):
                if step == first_step[qg] and qg > 0:
                    # batch the previous group's normalizations first
                    for jj in range(4):
                        norm_piece(qg - 1, jj)
                if t == 0 and par == 0:
                    pv = pvps.tile([P, 2, 512], f32, tag="pv", name=f"pv{qg}")
                    pv_tiles[qg] = pv[:, :, 0:510].rearrange(
                        "p b (i c) -> p b i c", c=85)
                pvr = pv_tiles[qg]
                q0b = max(0, t - 4 * qg)
                q0 = P * q0b
                stT, stH = sts[0]
                m = 2 * t + par
                pt_t = ptpool.tile([P, GH, QGS], bf16, tag="pt",
                                   name=f"pt{qg}_{m}")
                nc.scalar.activation(pt_t[:, 0:2, q0:], stT[:, :, q0:], Exp)
                with nc.allow_low_precision(
                        reason="schraudolph bf16 exp, ~3% on probs"):
                    nc.vector.tensor_scalar(
                        pt_t[:, 2, q0:].bitcast(mybir.dt.int16),
                        stH[:, q0:], A_SCH, B_SCH, mult, add)
                masked = t >= 4 * qg
                if masked:
                    pm = pt_t[:, :, q0:q0 + P]
                    nc.gpsimd.tensor_tensor(
                        pm, pm, msk_sb[:, par, :, :], mult)
                sts = [sts[1], emit_qk(step + 2)]
                jjs = list(range(q0b, 4))
                if masked and len(jjs) > 1:
                    jjs = jjs[1:] + [q0b]      # masked tile's PV last
                bank_started = set()
                for jj in jjs:
                    for h in range(GH):
                        b, isl = jj // 2, (jj % 2) * GH + h
                        first = (t == 0 and par == 0
                                 and b not in bank_started)
                        bank_started.add(b)
                        last = (t == 4 * qg + jj and par == 1)
                        nc.tensor.matmul(
                            pvr[:, b, isl, 0:65],
                            pt_t[:, h, jj * P:(jj + 1) * P],
                            vv[:, h, m, :],
                            start=first, stop=last,
                            skip_group_check=True)
                for fn in sched.get(step, []):
                    fn()
            for jj in range(4):
                norm_piece(QG - 1, jj)
                op_piece(QG - 1, jj)

    nc.compile()
    return nc


def _host_prep(inputs, Wq, bq, Wk, bk, Wv, bv, Wo, bo):
    import ml_dtypes

    bf16 = ml_dtypes.bfloat16
    e4 = ml_dtypes.float8_e4m3
    X = np.asarray(inputs, np.float32).reshape(S, D)
    XT = np.ascontiguousarray(X.T)                      # [768, 4096]

    def grp4(a, w, dt):  # [768, 4*w] -> [4, 128, 6*w]
        return np.ascontiguousarray(
            a.reshape(NC, P, QG, w).transpose(2, 1, 0, 3).reshape(QG, P, NC * w)
        ).astype(dt)

    xtb_h = grp4(XT, 1024, bf16)
    xt8_h = grp4(XT, 1024, e4)
    XTq = [np.ascontiguousarray(
        XT.reshape(D, NKB // 2, 2, P)[:, :, s_, :].reshape(D, SL))
        for s_ in range(2)]
    xq8_h = [grp4(x, 512, e4) for x in XTq]

    def wchunks(wt, scale, dt):  # [768, 192] -> [128, 1152]
        return np.ascontiguousarray(
            (wt * scale).reshape(NC, P, GD).transpose(1, 0, 2).reshape(P, NC * GD)
        ).astype(dt)

    # masks: [128, 2(par), 3(h), 128] int16, 0/-1;  par=0: s0 tri / s1 ones;
    # par=1: s0 zeros / s1 tri
    tri = np.where(np.arange(P)[None, :] >= np.arange(P)[:, None], 1.0, 0.0)
    ones = np.full((P, P), 1.0)
    zeros = np.zeros((P, P))
    mk = []
    for s_ in range(2):
        p0 = tri if s_ == 0 else ones
        p1 = zeros if s_ == 0 else tri
        m = np.stack([np.stack([p0] * GH), np.stack([p1] * GH)])  # [2,3,P,P]
        mk.append(np.ascontiguousarray(
            m.transpose(2, 0, 1, 3).reshape(P, 2 * GH * P)).astype(bf16))

    in_maps = []
    for g in range(NG):
        hs = slice(GD * g, GD * (g + 1))
        wq8_h = wchunks(np.ascontiguousarray(Wq[hs, :].T), 64.0, e4)
        wk8_h = wchunks(np.ascontiguousarray(Wk[hs, :].T), 64.0, e4)
        wvb_h = wchunks(np.ascontiguousarray(Wv[hs, :].T), 1.0, bf16)
        WoT = np.ascontiguousarray(Wo[:, hs].T).astype(np.float32)  # [192,768]
        bo_g = bv[hs].astype(np.float32) @ WoT
        if g == 0:
            bo_g = bo_g + bo.astype(np.float32)
        wota = np.concatenate([WoT, bo_g[None, :]], axis=0)  # [193, 768]
        wo0_h = np.ascontiguousarray(wota[0:P]).astype(bf16)
        wo1_h = np.ascontiguousarray(wota[P:]).astype(bf16)
        bq_h = np.zeros((P, 4), np.float32)
        bq_h[:, 0] = bq[hs][0:128] / 8.0
        bq_h[0:64, 1] = bq[hs][128:192] / 8.0
        bq_h[:, 2] = bk[hs][0:128]
        bq_h[0:64, 3] = bk[hs][128:192]
        for s_ in range(2):
            in_maps.append({
                "xtb": xtb_h, "xt8": xt8_h, "xq8": xq8_h[s_],
                "wq8": wq8_h, "wk8": wk8_h, "wvb": wvb_h,
                "wo0": wo0_h, "wo1": wo1_h, "bq4": bq_h, "mskp": mk[s_],
            })
    return in_maps


def _gather(results):
    NQB = SL // P
    out = np.zeros((S, D), np.float32)
    ov = out.reshape(NQB, 2, P, D)
    for s_ in range(2):
        acc = np.zeros((SL, D), np.float32)
        for g in range(NG):
            acc += np.asarray(results[2 * g + s_]["out"], np.float32)
        ov[:, s_, :, :] = acc.reshape(NQB, P, D)
    return out.reshape(1, S, D)


def kernel(inputs, Wq, bq, Wk, bk, Wv, bv, Wo, bo):
    from concourse.bass_utils import run_bass_kernel_spmd

    if "nc" not in _CACHE:
        _CACHE["nc"] = _build_program()
    nc = _CACHE["nc"]
    in_maps = _host_prep(
        np.asarray(inputs), np.asarray(Wq), np.asarray(bq), np.asarray(Wk),
        np.asarray(bk), np.asarray(Wv), np.asarray(bv), np.asarray(Wo),
        np.asarray(bo))
    res = run_bass_kernel_spmd(nc, in_maps, list(range(8))).results
    return _gather(res)


# revision 3
# speedup vs baseline: 1.0193x; 1.0193x over previous
"""Multi-head causal self-attention (D=768, H=12, S=4096) on 8 Trainium2 cores.

v2: q-major PV, fp8-DoubleRow Q/K projections, split exp (Act exact for
heads 0-1, DVE Schraudolph-bf16 for head 2), Pool int16 AND-masks,
DMA-transpose for the output-projection operand, sub-bank PSUM accumulators.

Sharding: 4 head-groups (3 heads) x 2 interleaved query-sets; core = 2g+s.
SPMD: one program, per-core behaviour via input data only.
"""

import numpy as np

D = 768
S = 4096
H = 12
HD = 64
NG = 4          # head groups
GH = 3          # heads per group
GD = GH * HD    # 192
SL = S // 2     # local queries per core
P = 128
NC = D // P     # 6 contraction chunks
QG = 4          # query groups per core
QGS = 512
NKB = S // P    # 32 key blocks

A_SCH = 128.0 / np.log(2.0)      # Schraudolph bf16 scale
B_SCH = 127.0 * 128.0 - 6.0      # bias, C=6 centering

_CACHE = {}


def _build_program():
    import concourse.bacc as bacc
    import concourse.mybir as mybir
    import concourse.tile as tile
    from contextlib import ExitStack

    bf16 = mybir.dt.bfloat16
    f32 = mybir.dt.float32
    fp8 = mybir.dt.float8e4
    i16 = mybir.dt.int16

    Exp = mybir.ActivationFunctionType.Exp
    Copy = mybir.ActivationFunctionType.Copy
    mult = mybir.AluOpType.mult
    add = mybir.AluOpType.add
    band = mybir.AluOpType.bitwise_and
    DR = mybir.MatmulPerfMode.DoubleRow

    nc = bacc.Bacc("TRN2", target_bir_lowering=False, debug=False, num_devices=8)

    xtb = nc.dram_tensor("xtb", [QG, P, NC * 1024], bf16, kind="ExternalInput").ap()
    xt8 = nc.dram_tensor("xt8", [QG, P, NC * 1024], fp8, kind="ExternalInput").ap()
    xq8 = nc.dram_tensor("xq8", [QG, P, NC * 512], fp8, kind="ExternalInput").ap()
    wq8 = nc.dram_tensor("wq8", [P, NC * GD], fp8, kind="ExternalInput").ap()
    wk8 = nc.dram_tensor("wk8", [P, NC * GD], fp8, kind="ExternalInput").ap()
    wvb = nc.dram_tensor("wvb", [P, NC * GD], bf16, kind="ExternalInput").ap()
    wo0 = nc.dram_tensor("wo0", [P, D], bf16, kind="ExternalInput").ap()
    wo1 = nc.dram_tensor("wo1", [65, D], bf16, kind="ExternalInput").ap()
    bq4 = nc.dram_tensor("bq4", [P, 4], f32, kind="ExternalInput").ap()
    mskp = nc.dram_tensor("mskp", [P, 2 * GH * P], bf16, kind="ExternalInput").ap()
    out = nc.dram_tensor("out", [SL, D], f32, kind="ExternalOutput").ap()
    import os as _os
    _dbg = _os.environ.get("KV2_DEBUG") == "1"
    if _dbg:
        dbg_qt = nc.dram_tensor("dbg_qt", [P, SL], bf16, kind="ExternalOutput").ap()
        dbg_kt = nc.dram_tensor("dbg_kt", [P, S], bf16, kind="ExternalOutput").ap()
        dbg_vv = nc.dram_tensor("dbg_vv", [P, GH * NKB * 65], bf16, kind="ExternalOutput").ap()
        dbg_s0 = nc.dram_tensor("dbg_s0", [P, QGS], bf16, kind="ExternalOutput").ap()
        dbg_s1 = nc.dram_tensor("dbg_s1", [P, QGS], bf16, kind="ExternalOutput").ap()
        dbg_pt = nc.dram_tensor("dbg_pt", [P, 2 * GH * QGS], bf16, kind="ExternalOutput").ap()
        dbg_den = nc.dram_tensor("dbg_den", [P, 12], f32, kind="ExternalOutput").ap()

    with tile.TileContext(nc) as tc, ExitStack() as ctx:
        const = ctx.enter_context(tc.tile_pool(name="const", bufs=1))

        xt_sb = const.tile([P, NC, S], bf16, tag="xt")
        xt8_sb = const.tile([P, NC, S], fp8, tag="xt8")
        xq8_sb = const.tile([P, NC, SL], fp8, tag="xq8")
        wq8_sb = const.tile([P, NC, GD], fp8, tag="wq8")
        wk8_sb = const.tile([P, NC, GD], fp8, tag="wk8")
        wv_sb = const.tile([P, NC, GD], bf16, tag="wv")
        wo0_sb = const.tile([P, D], bf16, tag="wo0")
        wo1_sb = const.tile([65, D], bf16, tag="wo1")
        bq_sb = const.tile([P, 4], f32, tag="bq")
        msk_sb = const.tile([P, 2, GH, P], bf16, tag="msk")
        kt01 = const.tile([P, S], bf16, tag="kt01")
        kt2 = const.tile([64, S], bf16, tag="kt2")
        qt01 = const.tile([P, SL], bf16, tag="qt01")
        qt2 = const.tile([64, SL], bf16, tag="qt2")
        vv = const.tile([P, GH, NKB, 65], bf16, tag="vv")
        stk0 = const.tile([P, QGS], bf16, tag="stk0")
        stk1 = const.tile([P, QGS], bf16, tag="stk1")

        # ---- input DMAs, priority order ----
        nc.sync.dma_start(wq8_sb[:], wq8[:].rearrange("p (c d) -> p c d", d=GD))
        nc.sync.dma_start(xq8_sb[:, :, 0:512],
                          xq8[0].rearrange("p (c d) -> p c d", d=512))
        nc.sync.dma_start(wk8_sb[:], wk8[:].rearrange("p (c d) -> p c d", d=GD))
        nc.sync.dma_start(xt8_sb[:, :, 0:1024],
                          xt8[0].rearrange("p (c d) -> p c d", d=1024))
        nc.sync.dma_start(wv_sb[:], wvb[:].rearrange("p (c d) -> p c d", d=GD))
        nc.sync.dma_start(xt_sb[:, :, 0:1024],
                          xtb[0].rearrange("p (c d) -> p c d", d=1024))
        nc.sync.dma_start(bq_sb[:], bq4[:])
        nc.sync.dma_start(msk_sb[:],
                          mskp[:].rearrange("p (a h q) -> p a h q", h=GH, q=P))
        nc.sync.dma_start(wo0_sb[:], wo0[:])
        nc.sync.dma_start(wo1_sb[:], wo1[:])
        for g in range(1, QG):
            nc.sync.dma_start(
                xq8_sb[:, :, g * 512:(g + 1) * 512],
                xq8[g].rearrange("p (c d) -> p c d", d=512))
            nc.sync.dma_start(
                xt8_sb[:, :, g * 1024:(g + 1) * 1024],
                xt8[g].rearrange("p (c d) -> p c d", d=1024))
            nc.sync.dma_start(
                xt_sb[:, :, g * 1024:(g + 1) * 1024],
                xtb[g].rearrange("p (c d) -> p c d", d=1024))
        nc.gpsimd.memset(vv[:], 1.0)         # ones column survives at [.., 64]

        kq = [(kt01, 0), (kt01, 64), (kt2, 0)]
        qq = [(qt01, 0), (qt01, 64), (qt2, 0)]

        # flattened block list: (qg, t, par), 80 blocks
        blocks = [(qg, t, par)
                  for qg in range(QG)
                  for t in range(4 * (qg + 1))
                  for par in (0, 1)]
        first_step = {}   # qg -> global step index of its first block
        for ii, (qg, t, par) in enumerate(blocks):
            if qg not in first_step:
                first_step[qg] = ii

        with tc.tile_pool(name="stA_ps", bufs=2, space="PSUM") as stA, \
             tc.tile_pool(name="stB_ps", bufs=2, space="PSUM") as stB, \
             tc.tile_pool(name="pv_ps", bufs=1, space="PSUM") as pvps, \
             tc.tile_pool(name="pt", bufs=4) as ptpool, \
             tc.tile_pool(name="sq", bufs=6) as sqpool, \
             tc.tile_pool(name="rc", bufs=8) as rcpool, \
             tc.tile_pool(name="oev", bufs=4) as oevpool:

            def proj_piece(g, idx):
                """Pieces 0..9 of projection chunk g (Q, K0, K1, V0..V3)."""
                if idx in (0, 1):
                    lo = idx == 0
                    ps = (stB.tile([P, QGS], f32, tag="stB", name=f"qA{g}")
                          if lo else
                          stB.tile([64, QGS], f32, tag="stB", name=f"qB{g}"))
                    dsl = slice(0, 128) if lo else slice(128, GD)
                    for j in range(3):
                        nc.tensor.matmul(
                            ps[:], wq8_sb[:, 2 * j:2 * j + 2, dsl],
                            xq8_sb[:, 2 * j:2 * j + 2, g * QGS:(g + 1) * QGS],
                            start=(j == 0), stop=(j == 2), perf_mode=DR)
                    if lo:
                        nc.vector.tensor_scalar(
                            qt01[:, g * QGS:(g + 1) * QGS], ps[:],
                            1.0 / 512.0, bq_sb[:, 0:1], mult, add)
                    else:
                        nc.vector.tensor_scalar(
                            qt2[:, g * QGS:(g + 1) * QGS], ps[:],
                            1.0 / 512.0, bq_sb[0:64, 1:2], mult, add)
                elif idx in (2, 3, 4, 5):
                    kg = 2 * g + (idx - 2) // 2
                    lo = (idx % 2) == 0
                    ps = (stB.tile([P, QGS], f32, tag="stB", name=f"kA{kg}")
                          if lo else
                          stB.tile([64, QGS], f32, tag="stB", name=f"kB{kg}"))
                    dsl = slice(0, 128) if lo else slice(128, GD)
                    for j in range(3):
                        nc.tensor.matmul(
                            ps[:], wk8_sb[:, 2 * j:2 * j + 2, dsl],
                            xt8_sb[:, 2 * j:2 * j + 2, kg * QGS:(kg + 1) * QGS],
                            start=(j == 0), stop=(j == 2), perf_mode=DR)
                    if lo:
                        nc.vector.tensor_scalar(
                            kt01[:, kg * QGS:(kg + 1) * QGS], ps[:],
                            1.0 / 64.0, bq_sb[:, 2:3], mult, add)
                    else:
                        nc.vector.tensor_scalar(
                            kt2[:, kg * QGS:(kg + 1) * QGS], ps[:],
                            1.0 / 64.0, bq_sb[0:64, 3:4], mult, add)
                else:
                    vp = idx - 6
                    kb0 = 8 * g + 2 * vp
                    psv = stB.tile([P, 2, GD], f32, tag="stB", name=f"v{kb0}")
                    for b in range(2):
                        for c in range(NC):
                            nc.tensor.matmul(
                                psv[:, b, :],
                                xt_sb[:, c, (kb0 + b) * P:(kb0 + b + 1) * P],
                                wv_sb[:, c, :],
                                start=(b == 0 and c == 0), stop=(c == NC - 1),
                                skip_group_check=True)
                    nc.vector.tensor_copy(
                        vv[:, :, kb0:kb0 + 2, 0:64],
                        psv[:].rearrange("p b (h d) -> p h b d", h=GH))

            pv_tiles = {}

            def emit_qk(step):
                if step >= len(blocks):
                    return None
                qg, t, par = blocks[step]
                q0 = P * max(0, t - 4 * qg)
                m = 2 * t + par
                stT = stA.tile([P, 2, QGS], f32, tag="stA",
                               name=f"sA{qg}_{m}")
                stH = stB.tile([P, QGS], f32, tag="stB", name=f"sB{qg}_{m}")
                for h in range(GH):
                    kt_t, kb_p = kq[h]
                    qt_t, qb_p = qq[h]
                    dst = stT[:, h, q0:] if h < 2 else stH[:, q0:]
                    nc.tensor.matmul(
                        dst,
                        kt_t[kb_p:kb_p + 64, m * P:(m + 1) * P],
                        qt_t[qb_p:qb_p + 64, qg * QGS + q0:(qg + 1) * QGS],
                        start=True, stop=True)
                return (stT, stH)

            def norm_piece(qg, jj):
                pvr = pv_tiles[qg]
                b, i0 = jj // 2, (jj % 2) * GH
                rc = rcpool.tile([P, GH, 1], f32, tag="rc",
                                 name=f"rc{qg}_{jj}")
                nc.vector.reciprocal(rc[:], pvr[:, b, i0:i0 + GH, 64:65])
                sq = sqpool.tile([P, 256], bf16, tag="sq",
                                 name=f"sq{qg}_{jj}")
                for h in range(GH):
                    nc.vector.tensor_scalar(
                        sq[:, h * 64:(h + 1) * 64],
                        pvr[:, b, i0 + h, 0:64], rc[:, h, :], None, mult)
                nc.vector.memset(sq[:, GD:GD + 1], 1.0)
                nc.sync.dma_start_transpose(
                    stk0[:, jj * P:(jj + 1) * P], sq[:, 0:128])
                nc.sync.dma_start_transpose(
                    stk1[:, jj * P:(jj + 1) * P], sq[:, 128:256])

            def op_piece(qg, jj):
                oe = oevpool.tile([P, D], f32, tag="oe", name=f"oe{qg}_{jj}")
                for half in range(2):
                    ps = stB.tile([P, QGS], f32, tag="stB",
                                  name=f"op{qg}_{jj}_{half}")
                    nc.tensor.matmul(
                        ps[:, 0:384],
                        stk0[:, jj * P:(jj + 1) * P],
                        wo0_sb[:, half * 384:(half + 1) * 384],
                        start=True, stop=False)
                    nc.tensor.matmul(
                        ps[:, 0:384],
                        stk1[0:65, jj * P:(jj + 1) * P],
                        wo1_sb[:, half * 384:(half + 1) * 384],
                        start=False, stop=True)
                    nc.vector.tensor_copy(
                        oe[:, half * 384:(half + 1) * 384], ps[:, 0:384])
                jq = 4 * qg + jj
                nc.sync.dma_start(out[jq * P:(jq + 1) * P, :], oe[:])

            # per-step extra pieces: step index -> list of callables
            sched = {}

            def add_piece(step, fn):
                sched.setdefault(step, []).append(fn)

            # chunk 0: Q, K0, V0 in prologue; K1, V1..V3 early steps
            add_piece(0, lambda: proj_piece(0, 4))
            add_piece(0, lambda: proj_piece(0, 5))
            add_piece(0, lambda: proj_piece(0, 7))   # V1 (blocks 2,3)
            add_piece(1, lambda: proj_piece(0, 8))   # V2
            add_piece(2, lambda: proj_piece(0, 9))   # V3
            # chunk qg+1 spread over qg's steps; tail of qg at start of qg+1
            for qg in range(QG):
                base = first_step[qg]
                nsteps = 8 * (qg + 1)
                if qg + 1 < QG:
                    for i in range(10):
                        add_piece(base + 3 + (i * (nsteps - 4)) // 10,
                                  lambda g=qg + 1, k=i: proj_piece(g, k))
                nbase = first_step[qg + 1] if qg + 1 < QG else len(blocks)
                for jj in range(4):
                    if qg + 1 < QG:
                        add_piece(nbase + 2 * jj,
                                  lambda g=qg, j=jj: op_piece(g, j))
                    # norms handled at boundary below

            # ---- prologue ----
            for i in (0, 1, 2, 3, 6):     # Q-A, Q-B, K0-A, K0-B, V0
                proj_piece(0, i)
            sts = [emit_qk(0), emit_qk(1)]

            # ---- main pipeline ----
            for step, (qg, t, par) in enumerate(blocks):
                if step == first_step[qg] and qg > 0:
                    # batch the previous group's normalizations first
                    for jj in range(4):
                        norm_piece(qg - 1, jj)
                if t == 0 and par == 0:
                    pv = pvps.tile([P, 2, 512], f32, tag="pv", name=f"pv{qg}")
                    pv_tiles[qg] = pv[:, :, 0:510].rearrange(
                        "p b (i c) -> p b i c", c=85)
                pvr = pv_tiles[qg]
                q0b = max(0, t - 4 * qg)
                q0 = P * q0b
                stT, stH = sts[0]
                m = 2 * t + par
                pt_t = ptpool.tile([P, GH, QGS], bf16, tag="pt",
                                   name=f"pt{qg}_{m}")
                nc.scalar.activation(pt_t[:, 0:2, q0:], stT[:, :, q0:], Exp)
                with nc.allow_low_precision(
                        reason="schraudolph bf16 exp, ~3% on probs"):
                    nc.vector.tensor_scalar(
                        pt_t[:, 2, q0:].bitcast(mybir.dt.int16),
                        stH[:, q0:], A_SCH, B_SCH, mult, add)
                masked = t >= 4 * qg
                if masked:
                    pm = pt_t[:, :, q0:q0 + P]
                    nc.vector.tensor_tensor(
                        pm, pm, msk_sb[:, par, :, :], mult)
                sts = [sts[1], emit_qk(step + 2)]
                jjs = list(range(q0b, 4))
                if masked and len(jjs) > 1:
                    jjs = jjs[1:] + [q0b]      # masked tile's PV last
                bank_started = set()
                for jj in jjs:
                    for h in range(GH):
                        b, isl = jj // 2, (jj % 2) * GH + h
                        first = (t == 0 and par == 0
                                 and b not in bank_started)
                        bank_started.add(b)
                        last = (t == 4 * qg + jj and par == 1)
                        nc.tensor.matmul(
                            pvr[:, b, isl, 0:65],
                            pt_t[:, h, jj * P:(jj + 1) * P],
                            vv[:, h, m, :],
                            start=first, stop=last,
                            skip_group_check=True)
                for fn in sched.get(step, []):
                    fn()
            for jj in range(4):
                norm_piece(QG - 1, jj)
                op_piece(QG - 1, jj)

    nc.compile()
    return nc


def _host_prep(inputs, Wq, bq, Wk, bk, Wv, bv, Wo, bo):
    import ml_dtypes

    bf16 = ml_dtypes.bfloat16
    e4 = ml_dtypes.float8_e4m3
    X = np.asarray(inputs, np.float32).reshape(S, D)
    XT = np.ascontiguousarray(X.T)                      # [768, 4096]

    def grp4(a, w, dt):  # [768, 4*w] -> [4, 128, 6*w]
        return np.ascontiguousarray(
            a.reshape(NC, P, QG, w).transpose(2, 1, 0, 3).reshape(QG, P, NC * w)
        ).astype(dt)

    xtb_h = grp4(XT, 1024, bf16)
    xt8_h = grp4(XT, 1024, e4)
    XTq = [np.ascontiguousarray(
        XT.reshape(D, NKB // 2, 2, P)[:, :, s_, :].reshape(D, SL))
        for s_ in range(2)]
    xq8_h = [grp4(x, 512, e4) for x in XTq]

    def wchunks(wt, scale, dt):  # [768, 192] -> [128, 1152]
        return np.ascontiguousarray(
            (wt * scale).reshape(NC, P, GD).transpose(1, 0, 2).reshape(P, NC * GD)
        ).astype(dt)

    # masks: [128, 2(par), 3(h), 128] int16, 0/-1;  par=0: s0 tri / s1 ones;
    # par=1: s0 zeros / s1 tri
    tri = np.where(np.arange(P)[None, :] >= np.arange(P)[:, None], 1.0, 0.0)
    ones = np.full((P, P), 1.0)
    zeros = np.zeros((P, P))
    mk = []
    for s_ in range(2):
        p0 = tri if s_ == 0 else ones
        p1 = zeros if s_ == 0 else tri
        m = np.stack([np.stack([p0] * GH), np.stack([p1] * GH)])  # [2,3,P,P]
        mk.append(np.ascontiguousarray(
            m.transpose(2, 0, 1, 3).reshape(P, 2 * GH * P)).astype(bf16))

    in_maps = []
    for g in range(NG):
        hs = slice(GD * g, GD * (g + 1))
        wq8_h = wchunks(np.ascontiguousarray(Wq[hs, :].T), 64.0, e4)
        wk8_h = wchunks(np.ascontiguousarray(Wk[hs, :].T), 64.0, e4)
        wvb_h = wchunks(np.ascontiguousarray(Wv[hs, :].T), 1.0, bf16)
        WoT = np.ascontiguousarray(Wo[:, hs].T).astype(np.float32)  # [192,768]
        bo_g = bv[hs].astype(np.float32) @ WoT
        if g == 0:
            bo_g = bo_g + bo.astype(np.float32)
        wota = np.concatenate([WoT, bo_g[None, :]], axis=0)  # [193, 768]
        wo0_h = np.ascontiguousarray(wota[0:P]).astype(bf16)
        wo1_h = np.ascontiguousarray(wota[P:]).astype(bf16)
        bq_h = np.zeros((P, 4), np.float32)
        bq_h[:, 0] = bq[hs][0:128] / 8.0
        bq_h[0:64, 1] = bq[hs][128:192] / 8.0
        bq_h[:, 2] = bk[hs][0:128]
        bq_h[0:64, 3] = bk[hs][128:192]
        for s_ in range(2):
            in_maps.append({
                "xtb": xtb_h, "xt8": xt8_h, "xq8": xq8_h[s_],
                "wq8": wq8_h, "wk8": wk8_h, "wvb": wvb_h,
                "wo0": wo0_h, "wo1": wo1_h, "bq4": bq_h, "mskp": mk[s_],
            })
    return in_maps


def _gather(results):
    NQB = SL // P
    out = np.zeros((S, D), np.float32)
    ov = out.reshape(NQB, 2, P, D)
    for s_ in range(2):
        acc = np.zeros((SL, D), np.float32)
        for g in range(NG):
            acc += np.asarray(results[2 * g + s_]["out"], np.float32)
        ov[:, s_, :, :] = acc.reshape(NQB, P, D)
    return out.reshape(1, S, D)


def kernel(inputs, Wq, bq, Wk, bk, Wv, bv, Wo, bo):
    from concourse.bass_utils import run_bass_kernel_spmd

    if "nc" not in _CACHE:
        _CACHE["nc"] = _build_program()
    nc = _CACHE["nc"]
    in_maps = _host_prep(
        np.asarray(inputs), np.asarray(Wq), np.asarray(bq), np.asarray(Wk),
        np.asarray(bk), np.asarray(Wv), np.asarray(bv), np.asarray(Wo),
        np.asarray(bo))
    res = run_bass_kernel_spmd(nc, in_maps, list(range(8))).results
    return _gather(res)


# revision 4
# speedup vs baseline: 1.0837x; 1.0632x over previous
"""Multi-head causal self-attention (D=768, H=12, S=4096) on 8 Trainium2 cores.

v2: q-major PV, fp8-DoubleRow Q/K projections, split exp (Act exact for
heads 0-1, DVE Schraudolph-bf16 for head 2), Pool int16 AND-masks,
DMA-transpose for the output-projection operand, sub-bank PSUM accumulators.

Sharding: 4 head-groups (3 heads) x 2 interleaved query-sets; core = 2g+s.
SPMD: one program, per-core behaviour via input data only.
"""

import numpy as np

D = 768
S = 4096
H = 12
HD = 64
NG = 4          # head groups
GH = 3          # heads per group
GD = GH * HD    # 192
SL = S // 2     # local queries per core
P = 128
NC = D // P     # 6 contraction chunks
QG = 4          # query groups per core
QGS = 512
NKB = S // P    # 32 key blocks

A_SCH = 128.0 / np.log(2.0)      # Schraudolph bf16 scale
B_SCH = 127.0 * 128.0 - 6.0      # bias, C=6 centering

_CACHE = {}


def _build_program():
    import concourse.bacc as bacc
    import concourse.mybir as mybir
    import concourse.tile as tile
    from contextlib import ExitStack

    bf16 = mybir.dt.bfloat16
    f32 = mybir.dt.float32
    fp8 = mybir.dt.float8e4
    i16 = mybir.dt.int16

    Exp = mybir.ActivationFunctionType.Exp
    Copy = mybir.ActivationFunctionType.Copy
    mult = mybir.AluOpType.mult
    add = mybir.AluOpType.add
    band = mybir.AluOpType.bitwise_and
    DR = mybir.MatmulPerfMode.DoubleRow

    nc = bacc.Bacc("TRN2", target_bir_lowering=False, debug=False, num_devices=8)

    xtb = nc.dram_tensor("xtb", [QG, P, NC * 1024], bf16, kind="ExternalInput").ap()
    xt8 = nc.dram_tensor("xt8", [QG, P, NC * 1024], fp8, kind="ExternalInput").ap()
    xq8 = nc.dram_tensor("xq8", [QG, P, NC * 512], fp8, kind="ExternalInput").ap()
    wq8 = nc.dram_tensor("wq8", [P, NC * GD], fp8, kind="ExternalInput").ap()
    wk8 = nc.dram_tensor("wk8", [P, NC * GD], fp8, kind="ExternalInput").ap()
    wvb = nc.dram_tensor("wvb", [P, NC * GD], bf16, kind="ExternalInput").ap()
    wo0 = nc.dram_tensor("wo0", [P, D], bf16, kind="ExternalInput").ap()
    wo1 = nc.dram_tensor("wo1", [65, D], bf16, kind="ExternalInput").ap()
    bq4 = nc.dram_tensor("bq4", [P, 4], f32, kind="ExternalInput").ap()
    mskp = nc.dram_tensor("mskp", [P, 2 * GH * P], bf16, kind="ExternalInput").ap()
    out = nc.dram_tensor("out", [SL, D], f32, kind="ExternalOutput").ap()
    import os as _os
    _dbg = _os.environ.get("KV2_DEBUG") == "1"
    if _dbg:
        dbg_qt = nc.dram_tensor("dbg_qt", [P, SL], bf16, kind="ExternalOutput").ap()
        dbg_kt = nc.dram_tensor("dbg_kt", [P, S], bf16, kind="ExternalOutput").ap()
        dbg_vv = nc.dram_tensor("dbg_vv", [P, GH * NKB * 65], bf16, kind="ExternalOutput").ap()
        dbg_s0 = nc.dram_tensor("dbg_s0", [P, QGS], bf16, kind="ExternalOutput").ap()
        dbg_s1 = nc.dram_tensor("dbg_s1", [P, QGS], bf16, kind="ExternalOutput").ap()
        dbg_pt = nc.dram_tensor("dbg_pt", [P, 2 * GH * QGS], bf16, kind="ExternalOutput").ap()
        dbg_den = nc.dram_tensor("dbg_den", [P, 12], f32, kind="ExternalOutput").ap()

    with tile.TileContext(nc) as tc, ExitStack() as ctx:
        const = ctx.enter_context(tc.tile_pool(name="const", bufs=1))

        xt_sb = const.tile([P, NC, S], bf16, tag="xt")
        xt8_sb = const.tile([P, NC, S], fp8, tag="xt8")
        xq8_sb = const.tile([P, NC, SL], fp8, tag="xq8")
        wq8_sb = const.tile([P, NC, GD], fp8, tag="wq8")
        wk8_sb = const.tile([P, NC, GD], fp8, tag="wk8")
        wv_sb = const.tile([P, NC, GD], bf16, tag="wv")
        wo0_sb = const.tile([P, D], bf16, tag="wo0")
        wo1_sb = const.tile([65, D], bf16, tag="wo1")
        bq_sb = const.tile([P, 4], f32, tag="bq")
        msk_sb = const.tile([P, 2, GH, P], bf16, tag="msk")
        kt01 = const.tile([P, S], bf16, tag="kt01")
        kt2 = const.tile([64, S], bf16, tag="kt2")
        qt01 = const.tile([P, SL], bf16, tag="qt01")
        qt2 = const.tile([64, SL], bf16, tag="qt2")
        vv = const.tile([P, GH, NKB, 65], bf16, tag="vv")
        stk0 = const.tile([P, QGS], bf16, tag="stk0")
        stk1 = const.tile([P, QGS], bf16, tag="stk1")

        # ---- input DMAs, priority order ----
        nc.sync.dma_start(wq8_sb[:], wq8[:].rearrange("p (c d) -> p c d", d=GD))
        nc.sync.dma_start(xq8_sb[:, :, 0:512],
                          xq8[0].rearrange("p (c d) -> p c d", d=512))
        nc.sync.dma_start(wk8_sb[:], wk8[:].rearrange("p (c d) -> p c d", d=GD))
        nc.sync.dma_start(xt8_sb[:, :, 0:1024],
                          xt8[0].rearrange("p (c d) -> p c d", d=1024))
        nc.sync.dma_start(wv_sb[:], wvb[:].rearrange("p (c d) -> p c d", d=GD))
        nc.sync.dma_start(xt_sb[:, :, 0:1024],
                          xtb[0].rearrange("p (c d) -> p c d", d=1024))
        nc.sync.dma_start(bq_sb[:], bq4[:])
        nc.sync.dma_start(msk_sb[:],
                          mskp[:].rearrange("p (a h q) -> p a h q", h=GH, q=P))
        nc.sync.dma_start(wo0_sb[:], wo0[:])
        nc.sync.dma_start(wo1_sb[:], wo1[:])
        for g in range(1, QG):
            nc.sync.dma_start(
                xq8_sb[:, :, g * 512:(g + 1) * 512],
                xq8[g].rearrange("p (c d) -> p c d", d=512))
            nc.sync.dma_start(
                xt8_sb[:, :, g * 1024:(g + 1) * 1024],
                xt8[g].rearrange("p (c d) -> p c d", d=1024))
            nc.sync.dma_start(
                xt_sb[:, :, g * 1024:(g + 1) * 1024],
                xtb[g].rearrange("p (c d) -> p c d", d=1024))
        nc.gpsimd.memset(vv[:], 1.0)         # ones column survives at [.., 64]

        kq = [(kt01, 0), (kt01, 64), (kt2, 0)]
        qq = [(qt01, 0), (qt01, 64), (qt2, 0)]

        # flattened block list: (qg, t, par), 80 blocks
        blocks = [(qg, t, par)
                  for qg in range(QG)
                  for t in range(4 * (qg + 1))
                  for par in (0, 1)]
        first_step = {}   # qg -> global step index of its first block
        for ii, (qg, t, par) in enumerate(blocks):
            if qg not in first_step:
                first_step[qg] = ii

        with tc.tile_pool(name="stA_ps", bufs=2, space="PSUM") as stA, \
             tc.tile_pool(name="stB_ps", bufs=2, space="PSUM") as stB, \
             tc.tile_pool(name="pv_ps", bufs=1, space="PSUM") as pvps, \
             tc.tile_pool(name="pt", bufs=12) as ptpool, \
             tc.tile_pool(name="sq", bufs=6) as sqpool, \
             tc.tile_pool(name="rc", bufs=8) as rcpool, \
             tc.tile_pool(name="oev", bufs=4) as oevpool:

            def proj_piece(g, idx):
                """Pieces 0..9 of projection chunk g (Q, K0, K1, V0..V3)."""
                if idx in (0, 1):
                    lo = idx == 0
                    ps = (stB.tile([P, QGS], f32, tag="stB", name=f"qA{g}")
                          if lo else
                          stB.tile([64, QGS], f32, tag="stB", name=f"qB{g}"))
                    dsl = slice(0, 128) if lo else slice(128, GD)
                    for j in range(3):
                        nc.tensor.matmul(
                            ps[:], wq8_sb[:, 2 * j:2 * j + 2, dsl],
                            xq8_sb[:, 2 * j:2 * j + 2, g * QGS:(g + 1) * QGS],
                            start=(j == 0), stop=(j == 2), perf_mode=DR)
                    if lo:
                        nc.vector.tensor_scalar(
                            qt01[:, g * QGS:(g + 1) * QGS], ps[:],
                            1.0 / 512.0, bq_sb[:, 0:1], mult, add)
                    else:
                        nc.vector.tensor_scalar(
                            qt2[:, g * QGS:(g + 1) * QGS], ps[:],
                            1.0 / 512.0, bq_sb[0:64, 1:2], mult, add)
                elif idx in (2, 3, 4, 5):
                    kg = 2 * g + (idx - 2) // 2
                    lo = (idx % 2) == 0
                    ps = (stB.tile([P, QGS], f32, tag="stB", name=f"kA{kg}")
                          if lo else
                          stB.tile([64, QGS], f32, tag="stB", name=f"kB{kg}"))
                    dsl = slice(0, 128) if lo else slice(128, GD)
                    for j in range(3):
                        nc.tensor.matmul(
                            ps[:], wk8_sb[:, 2 * j:2 * j + 2, dsl],
                            xt8_sb[:, 2 * j:2 * j + 2, kg * QGS:(kg + 1) * QGS],
                            start=(j == 0), stop=(j == 2), perf_mode=DR)
                    if lo:
                        nc.vector.tensor_scalar(
                            kt01[:, kg * QGS:(kg + 1) * QGS], ps[:],
                            1.0 / 64.0, bq_sb[:, 2:3], mult, add)
                    else:
                        nc.vector.tensor_scalar(
                            kt2[:, kg * QGS:(kg + 1) * QGS], ps[:],
                            1.0 / 64.0, bq_sb[0:64, 3:4], mult, add)
                else:
                    vp = idx - 6
                    kb0 = 8 * g + 2 * vp
                    psv = stB.tile([P, 2, GD], f32, tag="stB", name=f"v{kb0}")
                    for b in range(2):
                        for c in range(NC):
                            nc.tensor.matmul(
                                psv[:, b, :],
                                xt_sb[:, c, (kb0 + b) * P:(kb0 + b + 1) * P],
                                wv_sb[:, c, :],
                                start=(b == 0 and c == 0), stop=(c == NC - 1),
                                skip_group_check=True)
                    nc.vector.tensor_copy(
                        vv[:, :, kb0:kb0 + 2, 0:64],
                        psv[:].rearrange("p b (h d) -> p h b d", h=GH))

            pv_tiles = {}

            def emit_qk(step):
                if step >= len(blocks):
                    return None
                qg, t, par = blocks[step]
                q0 = P * max(0, t - 4 * qg)
                m = 2 * t + par
                stT = stA.tile([P, 2, QGS], f32, tag="stA",
                               name=f"sA{qg}_{m}")
                stH = stB.tile([P, QGS], f32, tag="stB", name=f"sB{qg}_{m}")
                for h in range(GH):
                    kt_t, kb_p = kq[h]
                    qt_t, qb_p = qq[h]
                    dst = stT[:, h, q0:] if h < 2 else stH[:, q0:]
                    nc.tensor.matmul(
                        dst,
                        kt_t[kb_p:kb_p + 64, m * P:(m + 1) * P],
                        qt_t[qb_p:qb_p + 64, qg * QGS + q0:(qg + 1) * QGS],
                        start=True, stop=True)
                return (stT, stH)

            def norm_piece(qg, jj):
                pvr = pv_tiles[qg]
                b, i0 = jj // 2, (jj % 2) * GH
                rc = rcpool.tile([P, GH, 1], f32, tag="rc",
                                 name=f"rc{qg}_{jj}")
                nc.vector.reciprocal(rc[:], pvr[:, b, i0:i0 + GH, 64:65])
                sq = sqpool.tile([P, 256], bf16, tag="sq",
                                 name=f"sq{qg}_{jj}")
                for h in range(GH):
                    nc.vector.tensor_scalar(
                        sq[:, h * 64:(h + 1) * 64],
                        pvr[:, b, i0 + h, 0:64], rc[:, h, :], None, mult)
                nc.vector.memset(sq[:, GD:GD + 1], 1.0)
                nc.sync.dma_start_transpose(
                    stk0[:, jj * P:(jj + 1) * P], sq[:, 0:128])
                nc.sync.dma_start_transpose(
                    stk1[:, jj * P:(jj + 1) * P], sq[:, 128:256])

            def op_piece(qg, jj):
                oe = oevpool.tile([P, D], f32, tag="oe", name=f"oe{qg}_{jj}")
                for half in range(2):
                    ps = stB.tile([P, QGS], f32, tag="stB",
                                  name=f"op{qg}_{jj}_{half}")
                    nc.tensor.matmul(
                        ps[:, 0:384],
                        stk0[:, jj * P:(jj + 1) * P],
                        wo0_sb[:, half * 384:(half + 1) * 384],
                        start=True, stop=False)
                    nc.tensor.matmul(
                        ps[:, 0:384],
                        stk1[0:65, jj * P:(jj + 1) * P],
                        wo1_sb[:, half * 384:(half + 1) * 384],
                        start=False, stop=True)
                    nc.vector.tensor_copy(
                        oe[:, half * 384:(half + 1) * 384], ps[:, 0:384])
                jq = 4 * qg + jj
                nc.sync.dma_start(out[jq * P:(jq + 1) * P, :], oe[:])

            # per-step extra pieces: step index -> list of callables
            sched = {}

            def add_piece(step, fn):
                sched.setdefault(step, []).append(fn)

            # chunk 0: Q, K0, V0 in prologue; K1, V1..V3 early steps
            add_piece(0, lambda: proj_piece(0, 4))
            add_piece(0, lambda: proj_piece(0, 5))
            add_piece(0, lambda: proj_piece(0, 7))   # V1 (blocks 2,3)
            add_piece(1, lambda: proj_piece(0, 8))   # V2
            add_piece(2, lambda: proj_piece(0, 9))   # V3
            # chunk qg+1 spread over qg's steps; tail of qg at start of qg+1
            for qg in range(QG):
                base = first_step[qg]
                nsteps = 8 * (qg + 1)
                if qg + 1 < QG:
                    for i in range(10):
                        add_piece(base + 3 + (i * (nsteps - 4)) // 10,
                                  lambda g=qg + 1, k=i: proj_piece(g, k))
                nbase = first_step[qg + 1] if qg + 1 < QG else len(blocks)
                for jj in range(4):
                    if qg + 1 < QG:
                        add_piece(nbase + 2 * jj,
                                  lambda g=qg, j=jj: op_piece(g, j))
                    # norms handled at boundary below

            # ---- prologue ----
            for i in (0, 1, 2, 3, 6):     # Q-A, Q-B, K0-A, K0-B, V0
                proj_piece(0, i)
            sts = [emit_qk(0), emit_qk(1)]

            # ---- main pipeline ----
            for step, (qg, t, par) in enumerate(blocks):
                if step == first_step[qg] and qg > 0:
                    # batch the previous group's normalizations first
                    for jj in range(4):
                        norm_piece(qg - 1, jj)
                if t == 0 and par == 0:
                    pv = pvps.tile([P, 2, 512], f32, tag="pv", name=f"pv{qg}")
                    pv_tiles[qg] = pv[:, :, 0:510].rearrange(
                        "p b (i c) -> p b i c", c=85)
                pvr = pv_tiles[qg]
                q0b = max(0, t - 4 * qg)
                q0 = P * q0b
                stT, stH = sts[0]
                m = 2 * t + par
                pt_t = ptpool.tile([P, GH, QGS], bf16, tag="pt",
                                   name=f"pt{qg}_{m}")
                nc.scalar.activation(pt_t[:, 0:2, q0:], stT[:, :, q0:], Exp)
                with nc.allow_low_precision(
                        reason="schraudolph bf16 exp, ~3% on probs"):
                    nc.vector.tensor_scalar(
                        pt_t[:, 2, q0:].bitcast(mybir.dt.int16),
                        stH[:, q0:], A_SCH, B_SCH, mult, add)
                masked = t >= 4 * qg
                if masked:
                    pm = pt_t[:, :, q0:q0 + P]
                    nc.vector.tensor_tensor(
                        pm, pm, msk_sb[:, par, :, :], mult)
                sts = [sts[1], emit_qk(step + 2)]
                jjs = list(range(q0b, 4))
                if masked and len(jjs) > 1:
                    jjs = jjs[1:] + [q0b]      # masked tile's PV last
                bank_started = set()
                for jj in jjs:
                    for h in range(GH):
                        b, isl = jj // 2, (jj % 2) * GH + h
                        first = (t == 0 and par == 0
                                 and b not in bank_started)
                        bank_started.add(b)
                        last = (t == 4 * qg + jj and par == 1)
                        nc.tensor.matmul(
                            pvr[:, b, isl, 0:65],
                            pt_t[:, h, jj * P:(jj + 1) * P],
                            vv[:, h, m, :],
                            start=first, stop=last,
                            skip_group_check=True)
                for fn in sched.get(step, []):
                    fn()
            for jj in range(4):
                norm_piece(QG - 1, jj)
                op_piece(QG - 1, jj)

    nc.compile()
    return nc


def _host_prep(inputs, Wq, bq, Wk, bk, Wv, bv, Wo, bo):
    import ml_dtypes

    bf16 = ml_dtypes.bfloat16
    e4 = ml_dtypes.float8_e4m3
    X = np.asarray(inputs, np.float32).reshape(S, D)
    XT = np.ascontiguousarray(X.T)                      # [768, 4096]

    def grp4(a, w, dt):  # [768, 4*w] -> [4, 128, 6*w]
        return np.ascontiguousarray(
            a.reshape(NC, P, QG, w).transpose(2, 1, 0, 3).reshape(QG, P, NC * w)
        ).astype(dt)

    xtb_h = grp4(XT, 1024, bf16)
    xt8_h = grp4(XT, 1024, e4)
    XTq = [np.ascontiguousarray(
        XT.reshape(D, NKB // 2, 2, P)[:, :, s_, :].reshape(D, SL))
        for s_ in range(2)]
    xq8_h = [grp4(x, 512, e4) for x in XTq]

    def wchunks(wt, scale, dt):  # [768, 192] -> [128, 1152]
        return np.ascontiguousarray(
            (wt * scale).reshape(NC, P, GD).transpose(1, 0, 2).reshape(P, NC * GD)
        ).astype(dt)

    # masks: [128, 2(par), 3(h), 128] int16, 0/-1;  par=0: s0 tri / s1 ones;
    # par=1: s0 zeros / s1 tri
    tri = np.where(np.arange(P)[None, :] >= np.arange(P)[:, None], 1.0, 0.0)
    ones = np.full((P, P), 1.0)
    zeros = np.zeros((P, P))
    mk = []
    for s_ in range(2):
        p0 = tri if s_ == 0 else ones
        p1 = zeros if s_ == 0 else tri
        m = np.stack([np.stack([p0] * GH), np.stack([p1] * GH)])  # [2,3,P,P]
        mk.append(np.ascontiguousarray(
            m.transpose(2, 0, 1, 3).reshape(P, 2 * GH * P)).astype(bf16))

    in_maps = []
    for g in range(NG):
        hs = slice(GD * g, GD * (g + 1))
        wq8_h = wchunks(np.ascontiguousarray(Wq[hs, :].T), 64.0, e4)
        wk8_h = wchunks(np.ascontiguousarray(Wk[hs, :].T), 64.0, e4)
        wvb_h = wchunks(np.ascontiguousarray(Wv[hs, :].T), 1.0, bf16)
        WoT = np.ascontiguousarray(Wo[:, hs].T).astype(np.float32)  # [192,768]
        bo_g = bv[hs].astype(np.float32) @ WoT
        if g == 0:
            bo_g = bo_g + bo.astype(np.float32)
        wota = np.concatenate([WoT, bo_g[None, :]], axis=0)  # [193, 768]
        wo0_h = np.ascontiguousarray(wota[0:P]).astype(bf16)
        wo1_h = np.ascontiguousarray(wota[P:]).astype(bf16)
        bq_h = np.zeros((P, 4), np.float32)
        bq_h[:, 0] = bq[hs][0:128] / 8.0
        bq_h[0:64, 1] = bq[hs][128:192] / 8.0
        bq_h[:, 2] = bk[hs][0:128]
        bq_h[0:64, 3] = bk[hs][128:192]
        for s_ in range(2):
            in_maps.append({
                "xtb": xtb_h, "xt8": xt8_h, "xq8": xq8_h[s_],
                "wq8": wq8_h, "wk8": wk8_h, "wvb": wvb_h,
                "wo0": wo0_h, "wo1": wo1_h, "bq4": bq_h, "mskp": mk[s_],
            })
    return in_maps


def _gather(results):
    NQB = SL // P
    out = np.zeros((S, D), np.float32)
    ov = out.reshape(NQB, 2, P, D)
    for s_ in range(2):
        acc = np.zeros((SL, D), np.float32)
        for g in range(NG):
            acc += np.asarray(results[2 * g + s_]["out"], np.float32)
        ov[:, s_, :, :] = acc.reshape(NQB, P, D)
    return out.reshape(1, S, D)


def kernel(inputs, Wq, bq, Wk, bk, Wv, bv, Wo, bo):
    from concourse.bass_utils import run_bass_kernel_spmd

    if "nc" not in _CACHE:
        _CACHE["nc"] = _build_program()
    nc = _CACHE["nc"]
    in_maps = _host_prep(
        np.asarray(inputs), np.asarray(Wq), np.asarray(bq), np.asarray(Wk),
        np.asarray(bk), np.asarray(Wv), np.asarray(bv), np.asarray(Wo),
        np.asarray(bo))
    res = run_bass_kernel_spmd(nc, in_maps, list(range(8))).results
    return _gather(res)


# revision 5
# speedup vs baseline: 1.1000x; 1.0151x over previous
"""Multi-head causal self-attention (D=768, H=12, S=4096) on 8 Trainium2 cores.

v2: q-major PV, fp8-DoubleRow Q/K projections, split exp (Act exact for
heads 0-1, DVE Schraudolph-bf16 for head 2), Pool int16 AND-masks,
DMA-transpose for the output-projection operand, sub-bank PSUM accumulators.

Sharding: 4 head-groups (3 heads) x 2 interleaved query-sets; core = 2g+s.
SPMD: one program, per-core behaviour via input data only.
"""

import numpy as np

D = 768
S = 4096
H = 12
HD = 64
NG = 4          # head groups
GH = 3          # heads per group
GD = GH * HD    # 192
SL = S // 2     # local queries per core
P = 128
NC = D // P     # 6 contraction chunks
QG = 4          # query groups per core
QGS = 512
NKB = S // P    # 32 key blocks

A_SCH = 128.0 / np.log(2.0)      # Schraudolph bf16 scale
B_SCH = 127.0 * 128.0 - 6.0      # bias, C=6 centering

_CACHE = {}


def _build_program():
    import concourse.bacc as bacc
    import concourse.mybir as mybir
    import concourse.tile as tile
    from contextlib import ExitStack

    bf16 = mybir.dt.bfloat16
    f32 = mybir.dt.float32
    fp8 = mybir.dt.float8e4
    i16 = mybir.dt.int16

    Exp = mybir.ActivationFunctionType.Exp
    Copy = mybir.ActivationFunctionType.Copy
    mult = mybir.AluOpType.mult
    add = mybir.AluOpType.add
    band = mybir.AluOpType.bitwise_and
    DR = mybir.MatmulPerfMode.DoubleRow

    nc = bacc.Bacc("TRN2", target_bir_lowering=False, debug=False, num_devices=8)

    xtb = nc.dram_tensor("xtb", [QG, P, NC * 1024], bf16, kind="ExternalInput").ap()
    xt8 = nc.dram_tensor("xt8", [QG, P, NC * 1024], fp8, kind="ExternalInput").ap()
    xq8 = nc.dram_tensor("xq8", [QG, P, NC * 512], fp8, kind="ExternalInput").ap()
    wq8 = nc.dram_tensor("wq8", [P, NC * GD], fp8, kind="ExternalInput").ap()
    wk8 = nc.dram_tensor("wk8", [P, NC * GD], fp8, kind="ExternalInput").ap()
    wvb = nc.dram_tensor("wvb", [P, NC * GD], bf16, kind="ExternalInput").ap()
    wo0 = nc.dram_tensor("wo0", [P, D], bf16, kind="ExternalInput").ap()
    wo1 = nc.dram_tensor("wo1", [65, D], bf16, kind="ExternalInput").ap()
    bq4 = nc.dram_tensor("bq4", [P, 4], f32, kind="ExternalInput").ap()
    mskp = nc.dram_tensor("mskp", [P, 2 * GH * P], bf16, kind="ExternalInput").ap()
    out = nc.dram_tensor("out", [SL, D], f32, kind="ExternalOutput").ap()
    import os as _os
    _dbg = _os.environ.get("KV2_DEBUG") == "1"
    if _dbg:
        dbg_qt = nc.dram_tensor("dbg_qt", [P, SL], bf16, kind="ExternalOutput").ap()
        dbg_kt = nc.dram_tensor("dbg_kt", [P, S], bf16, kind="ExternalOutput").ap()
        dbg_vv = nc.dram_tensor("dbg_vv", [P, GH * NKB * 65], bf16, kind="ExternalOutput").ap()
        dbg_s0 = nc.dram_tensor("dbg_s0", [P, QGS], bf16, kind="ExternalOutput").ap()
        dbg_s1 = nc.dram_tensor("dbg_s1", [P, QGS], bf16, kind="ExternalOutput").ap()
        dbg_pt = nc.dram_tensor("dbg_pt", [P, 2 * GH * QGS], bf16, kind="ExternalOutput").ap()
        dbg_den = nc.dram_tensor("dbg_den", [P, 12], f32, kind="ExternalOutput").ap()

    with tile.TileContext(nc) as tc, ExitStack() as ctx:
        const = ctx.enter_context(tc.tile_pool(name="const", bufs=1))

        xt_sb = const.tile([P, NC, S], bf16, tag="xt")
        xt8_sb = const.tile([P, NC, S], fp8, tag="xt8")
        xq8_sb = const.tile([P, NC, SL], fp8, tag="xq8")
        wq8_sb = const.tile([P, NC, GD], fp8, tag="wq8")
        wk8_sb = const.tile([P, NC, GD], fp8, tag="wk8")
        wv_sb = const.tile([P, NC, GD], bf16, tag="wv")
        wo0_sb = const.tile([P, D], bf16, tag="wo0")
        wo1_sb = const.tile([65, D], bf16, tag="wo1")
        bq_sb = const.tile([P, 4], f32, tag="bq")
        msk_sb = const.tile([P, 2, GH, P], bf16, tag="msk")
        kt01 = const.tile([P, S], bf16, tag="kt01")
        kt2 = const.tile([64, S], bf16, tag="kt2")
        qt01 = const.tile([P, SL], bf16, tag="qt01")
        qt2 = const.tile([64, SL], bf16, tag="qt2")
        vv = const.tile([P, GH, NKB, 65], bf16, tag="vv")
        stk0 = const.tile([P, QGS], bf16, tag="stk0")
        stk1 = const.tile([P, QGS], bf16, tag="stk1")

        # ---- input DMAs, priority order ----
        nc.sync.dma_start(wq8_sb[:], wq8[:].rearrange("p (c d) -> p c d", d=GD))
        nc.sync.dma_start(xq8_sb[:, :, 0:512],
                          xq8[0].rearrange("p (c d) -> p c d", d=512))
        nc.sync.dma_start(wk8_sb[:], wk8[:].rearrange("p (c d) -> p c d", d=GD))
        nc.sync.dma_start(xt8_sb[:, :, 0:1024],
                          xt8[0].rearrange("p (c d) -> p c d", d=1024))
        nc.sync.dma_start(wv_sb[:], wvb[:].rearrange("p (c d) -> p c d", d=GD))
        xtb0 = xtb[0].rearrange("p (c d) -> p c d", d=1024)
        nc.sync.dma_start(xt_sb[:, :, 0:512], xtb0[:, :, 0:512])
        nc.sync.dma_start(xt_sb[:, :, 512:1024], xtb0[:, :, 512:1024])
        nc.sync.dma_start(bq_sb[:], bq4[:])
        nc.sync.dma_start(msk_sb[:],
                          mskp[:].rearrange("p (a h q) -> p a h q", h=GH, q=P))
        nc.sync.dma_start(wo0_sb[:], wo0[:])
        nc.sync.dma_start(wo1_sb[:], wo1[:])
        for g in range(1, QG):
            nc.sync.dma_start(
                xq8_sb[:, :, g * 512:(g + 1) * 512],
                xq8[g].rearrange("p (c d) -> p c d", d=512))
            nc.sync.dma_start(
                xt8_sb[:, :, g * 1024:(g + 1) * 1024],
                xt8[g].rearrange("p (c d) -> p c d", d=1024))
            nc.sync.dma_start(
                xt_sb[:, :, g * 1024:(g + 1) * 1024],
                xtb[g].rearrange("p (c d) -> p c d", d=1024))
        nc.gpsimd.memset(vv[:], 1.0)         # ones column survives at [.., 64]

        kq = [(kt01, 0), (kt01, 64), (kt2, 0)]
        qq = [(qt01, 0), (qt01, 64), (qt2, 0)]

        # flattened block list: (qg, t, par), 80 blocks
        blocks = [(qg, t, par)
                  for qg in range(QG)
                  for t in range(4 * (qg + 1))
                  for par in (0, 1)]
        first_step = {}   # qg -> global step index of its first block
        for ii, (qg, t, par) in enumerate(blocks):
            if qg not in first_step:
                first_step[qg] = ii

        with tc.tile_pool(name="stA_ps", bufs=2, space="PSUM") as stA, \
             tc.tile_pool(name="stB_ps", bufs=2, space="PSUM") as stB, \
             tc.tile_pool(name="pv_ps", bufs=1, space="PSUM") as pvps, \
             tc.tile_pool(name="pt", bufs=12) as ptpool, \
             tc.tile_pool(name="sq", bufs=6) as sqpool, \
             tc.tile_pool(name="rc", bufs=8) as rcpool, \
             tc.tile_pool(name="oev", bufs=4) as oevpool:

            def proj_piece(g, idx):
                """Pieces 0..9 of projection chunk g (Q, K0, K1, V0..V3)."""
                if idx in (0, 1):
                    lo = idx == 0
                    ps = (stB.tile([P, QGS], f32, tag="stB", name=f"qA{g}")
                          if lo else
                          stB.tile([64, QGS], f32, tag="stB", name=f"qB{g}"))
                    dsl = slice(0, 128) if lo else slice(128, GD)
                    for j in range(3):
                        nc.tensor.matmul(
                            ps[:], wq8_sb[:, 2 * j:2 * j + 2, dsl],
                            xq8_sb[:, 2 * j:2 * j + 2, g * QGS:(g + 1) * QGS],
                            start=(j == 0), stop=(j == 2), perf_mode=DR)
                    if lo:
                        nc.vector.tensor_scalar(
                            qt01[:, g * QGS:(g + 1) * QGS], ps[:],
                            1.0 / 512.0, bq_sb[:, 0:1], mult, add)
                    else:
                        nc.vector.tensor_scalar(
                            qt2[:, g * QGS:(g + 1) * QGS], ps[:],
                            1.0 / 512.0, bq_sb[0:64, 1:2], mult, add)
                elif idx in (2, 3, 4, 5):
                    kg = 2 * g + (idx - 2) // 2
                    lo = (idx % 2) == 0
                    ps = (stB.tile([P, QGS], f32, tag="stB", name=f"kA{kg}")
                          if lo else
                          stB.tile([64, QGS], f32, tag="stB", name=f"kB{kg}"))
                    dsl = slice(0, 128) if lo else slice(128, GD)
                    for j in range(3):
                        nc.tensor.matmul(
                            ps[:], wk8_sb[:, 2 * j:2 * j + 2, dsl],
                            xt8_sb[:, 2 * j:2 * j + 2, kg * QGS:(kg + 1) * QGS],
                            start=(j == 0), stop=(j == 2), perf_mode=DR)
                    if lo:
                        nc.vector.tensor_scalar(
                            kt01[:, kg * QGS:(kg + 1) * QGS], ps[:],
                            1.0 / 64.0, bq_sb[:, 2:3], mult, add)
                    else:
                        nc.vector.tensor_scalar(
                            kt2[:, kg * QGS:(kg + 1) * QGS], ps[:],
                            1.0 / 64.0, bq_sb[0:64, 3:4], mult, add)
                else:
                    vp = idx - 6
                    kb0 = 8 * g + 2 * vp
                    psv = stB.tile([P, 2, GD], f32, tag="stB", name=f"v{kb0}")
                    for b in range(2):
                        for c in range(NC):
                            nc.tensor.matmul(
                                psv[:, b, :],
                                xt_sb[:, c, (kb0 + b) * P:(kb0 + b + 1) * P],
                                wv_sb[:, c, :],
                                start=(b == 0 and c == 0), stop=(c == NC - 1),
                                skip_group_check=True)
                    nc.vector.tensor_copy(
                        vv[:, :, kb0:kb0 + 2, 0:64],
                        psv[:].rearrange("p b (h d) -> p h b d", h=GH))

            pv_tiles = {}

            def emit_qk(step):
                if step >= len(blocks):
                    return None
                qg, t, par = blocks[step]
                q0 = P * max(0, t - 4 * qg)
                m = 2 * t + par
                stT = stA.tile([P, 2, QGS], f32, tag="stA",
                               name=f"sA{qg}_{m}")
                stH = stB.tile([P, QGS], f32, tag="stB", name=f"sB{qg}_{m}")
                for h in range(GH):
                    kt_t, kb_p = kq[h]
                    qt_t, qb_p = qq[h]
                    dst = stT[:, h, q0:] if h < 2 else stH[:, q0:]
                    nc.tensor.matmul(
                        dst,
                        kt_t[kb_p:kb_p + 64, m * P:(m + 1) * P],
                        qt_t[qb_p:qb_p + 64, qg * QGS + q0:(qg + 1) * QGS],
                        start=True, stop=True)
                return (stT, stH)

            def norm_piece(qg, jj):
                pvr = pv_tiles[qg]
                b, i0 = jj // 2, (jj % 2) * GH
                rc = rcpool.tile([P, GH, 1], f32, tag="rc",
                                 name=f"rc{qg}_{jj}")
                nc.vector.reciprocal(rc[:], pvr[:, b, i0:i0 + GH, 64:65])
                sq = sqpool.tile([P, 256], bf16, tag="sq",
                                 name=f"sq{qg}_{jj}")
                for h in range(GH):
                    nc.vector.tensor_scalar(
                        sq[:, h * 64:(h + 1) * 64],
                        pvr[:, b, i0 + h, 0:64], rc[:, h, :], None, mult)
                nc.vector.memset(sq[:, GD:GD + 1], 1.0)
                nc.sync.dma_start_transpose(
                    stk0[:, jj * P:(jj + 1) * P], sq[:, 0:128])
                nc.sync.dma_start_transpose(
                    stk1[:, jj * P:(jj + 1) * P], sq[:, 128:256])

            def op_piece(qg, jj):
                oe = oevpool.tile([P, D], f32, tag="oe", name=f"oe{qg}_{jj}")
                for half in range(2):
                    ps = stB.tile([P, QGS], f32, tag="stB",
                                  name=f"op{qg}_{jj}_{half}")
                    nc.tensor.matmul(
                        ps[:, 0:384],
                        stk0[:, jj * P:(jj + 1) * P],
                        wo0_sb[:, half * 384:(half + 1) * 384],
                        start=True, stop=False)
                    nc.tensor.matmul(
                        ps[:, 0:384],
                        stk1[0:65, jj * P:(jj + 1) * P],
                        wo1_sb[:, half * 384:(half + 1) * 384],
                        start=False, stop=True)
                    nc.vector.tensor_copy(
                        oe[:, half * 384:(half + 1) * 384], ps[:, 0:384])
                jq = 4 * qg + jj
                nc.sync.dma_start(out[jq * P:(jq + 1) * P, :], oe[:])

            # per-step extra pieces: step index -> list of callables
            sched = {}

            def add_piece(step, fn):
                sched.setdefault(step, []).append(fn)

            # chunk 0: Q, K0, V0 in prologue; K1, V1..V3 early steps
            add_piece(0, lambda: proj_piece(0, 4))
            add_piece(0, lambda: proj_piece(0, 5))
            add_piece(0, lambda: proj_piece(0, 7))   # V1 (blocks 2,3)
            add_piece(1, lambda: proj_piece(0, 8))   # V2
            add_piece(2, lambda: proj_piece(0, 9))   # V3
            # chunk qg+1 spread over qg's steps; tail of qg at start of qg+1
            for qg in range(QG):
                base = first_step[qg]
                nsteps = 8 * (qg + 1)
                if qg + 1 < QG:
                    for i in range(10):
                        add_piece(base + 3 + (i * (nsteps - 4)) // 10,
                                  lambda g=qg + 1, k=i: proj_piece(g, k))
                nbase = first_step[qg + 1] if qg + 1 < QG else len(blocks)
                for jj in range(4):
                    if qg + 1 < QG:
                        stop_step = first_step[qg] + 2 * (4 * qg + jj) + 1
                        add_piece(stop_step + 1,
                                  lambda g=qg, j=jj: norm_piece(g, j))
                        add_piece(nbase + 2 * jj,
                                  lambda g=qg, j=jj: op_piece(g, j))
                    else:
                        stop_step = first_step[qg] + 2 * (4 * qg + jj) + 1
                        add_piece(stop_step + 1,
                                  lambda g=qg, j=jj: norm_piece(g, j))
                        add_piece(stop_step + 2,
                                  lambda g=qg, j=jj: op_piece(g, j))

            # ---- prologue ----
            for i in (0, 1, 2, 3, 6):     # Q-A, Q-B, K0-A, K0-B, V0
                proj_piece(0, i)
            sts = [emit_qk(0), emit_qk(1)]

            # ---- main pipeline ----
            for step, (qg, t, par) in enumerate(blocks):
                if t == 0 and par == 0:
                    pv = pvps.tile([P, 2, 512], f32, tag="pv", name=f"pv{qg}")
                    pv_tiles[qg] = pv[:, :, 0:510].rearrange(
                        "p b (i c) -> p b i c", c=85)
                pvr = pv_tiles[qg]
                q0b = max(0, t - 4 * qg)
                q0 = P * q0b
                stT, stH = sts[0]
                m = 2 * t + par
                pt_t = ptpool.tile([P, GH, QGS], bf16, tag="pt",
                                   name=f"pt{qg}_{m}")
                nc.scalar.activation(pt_t[:, 0:2, q0:], stT[:, :, q0:], Exp)
                with nc.allow_low_precision(
                        reason="schraudolph bf16 exp, ~3% on probs"):
                    nc.vector.tensor_scalar(
                        pt_t[:, 2, q0:].bitcast(mybir.dt.int16),
                        stH[:, q0:], A_SCH, B_SCH, mult, add)
                masked = t >= 4 * qg
                if masked:
                    pm = pt_t[:, :, q0:q0 + P]
                    nc.vector.tensor_tensor(
                        pm, pm, msk_sb[:, par, :, :], mult)
                sts = [sts[1], emit_qk(step + 2)]
                jjs = list(range(q0b, 4))
                if masked and len(jjs) > 1:
                    jjs = jjs[1:] + [q0b]      # masked tile's PV last
                bank_started = set()
                for jj in jjs:
                    for h in range(GH):
                        b, isl = jj // 2, (jj % 2) * GH + h
                        first = (t == 0 and par == 0
                                 and b not in bank_started)
                        bank_started.add(b)
                        last = (t == 4 * qg + jj and par == 1)
                        nc.tensor.matmul(
                            pvr[:, b, isl, 0:65],
                            pt_t[:, h, jj * P:(jj + 1) * P],
                            vv[:, h, m, :],
                            start=first, stop=last,
                            skip_group_check=True)
                for fn in sched.get(step, []):
                    fn()
            for step in range(len(blocks), len(blocks) + 4):
                for fn in sched.get(step, []):
                    fn()

    nc.compile()
    return nc


def _host_prep(inputs, Wq, bq, Wk, bk, Wv, bv, Wo, bo):
    import ml_dtypes

    bf16 = ml_dtypes.bfloat16
    e4 = ml_dtypes.float8_e4m3
    X = np.asarray(inputs, np.float32).reshape(S, D)
    XT = np.ascontiguousarray(X.T)                      # [768, 4096]

    def grp4(a, w, dt):  # [768, 4*w] -> [4, 128, 6*w]
        return np.ascontiguousarray(
            a.reshape(NC, P, QG, w).transpose(2, 1, 0, 3).reshape(QG, P, NC * w)
        ).astype(dt)

    xtb_h = grp4(XT, 1024, bf16)
    xt8_h = grp4(XT, 1024, e4)
    XTq = [np.ascontiguousarray(
        XT.reshape(D, NKB // 2, 2, P)[:, :, s_, :].reshape(D, SL))
        for s_ in range(2)]
    xq8_h = [grp4(x, 512, e4) for x in XTq]

    def wchunks(wt, scale, dt):  # [768, 192] -> [128, 1152]
        return np.ascontiguousarray(
            (wt * scale).reshape(NC, P, GD).transpose(1, 0, 2).reshape(P, NC * GD)
        ).astype(dt)

    # masks: [128, 2(par), 3(h), 128] int16, 0/-1;  par=0: s0 tri / s1 ones;
    # par=1: s0 zeros / s1 tri
    tri = np.where(np.arange(P)[None, :] >= np.arange(P)[:, None], 1.0, 0.0)
    ones = np.full((P, P), 1.0)
    zeros = np.zeros((P, P))
    mk = []
    for s_ in range(2):
        p0 = tri if s_ == 0 else ones
        p1 = zeros if s_ == 0 else tri
        m = np.stack([np.stack([p0] * GH), np.stack([p1] * GH)])  # [2,3,P,P]
        mk.append(np.ascontiguousarray(
            m.transpose(2, 0, 1, 3).reshape(P, 2 * GH * P)).astype(bf16))

    in_maps = []
    for g in range(NG):
        hs = slice(GD * g, GD * (g + 1))
        wq8_h = wchunks(np.ascontiguousarray(Wq[hs, :].T), 64.0, e4)
        wk8_h = wchunks(np.ascontiguousarray(Wk[hs, :].T), 64.0, e4)
        wvb_h = wchunks(np.ascontiguousarray(Wv[hs, :].T), 1.0, bf16)
        WoT = np.ascontiguousarray(Wo[:, hs].T).astype(np.float32)  # [192,768]
        bo_g = bv[hs].astype(np.float32) @ WoT
        if g == 0:
            bo_g = bo_g + bo.astype(np.float32)
        wota = np.concatenate([WoT, bo_g[None, :]], axis=0)  # [193, 768]
        wo0_h = np.ascontiguousarray(wota[0:P]).astype(bf16)
        wo1_h = np.ascontiguousarray(wota[P:]).astype(bf16)
        bq_h = np.zeros((P, 4), np.float32)
        bq_h[:, 0] = bq[hs][0:128] / 8.0
        bq_h[0:64, 1] = bq[hs][128:192] / 8.0
        bq_h[:, 2] = bk[hs][0:128]
        bq_h[0:64, 3] = bk[hs][128:192]
        for s_ in range(2):
            in_maps.append({
                "xtb": xtb_h, "xt8": xt8_h, "xq8": xq8_h[s_],
                "wq8": wq8_h, "wk8": wk8_h, "wvb": wvb_h,
                "wo0": wo0_h, "wo1": wo1_h, "bq4": bq_h, "mskp": mk[s_],
            })
    return in_maps


def _gather(results):
    NQB = SL // P
    out = np.zeros((S, D), np.float32)
    ov = out.reshape(NQB, 2, P, D)
    for s_ in range(2):
        acc = np.zeros((SL, D), np.float32)
        for g in range(NG):
            acc += np.asarray(results[2 * g + s_]["out"], np.float32)
        ov[:, s_, :, :] = acc.reshape(NQB, P, D)
    return out.reshape(1, S, D)


def kernel(inputs, Wq, bq, Wk, bk, Wv, bv, Wo, bo):
    from concourse.bass_utils import run_bass_kernel_spmd

    if "nc" not in _CACHE:
        _CACHE["nc"] = _build_program()
    nc = _CACHE["nc"]
    in_maps = _host_prep(
        np.asarray(inputs), np.asarray(Wq), np.asarray(bq), np.asarray(Wk),
        np.asarray(bk), np.asarray(Wv), np.asarray(bv), np.asarray(Wo),
        np.asarray(bo))
    res = run_bass_kernel_spmd(nc, in_maps, list(range(8))).results
    return _gather(res)
